# revision 6
# baseline (speedup 1.0000x reference)
"""Causal self-attention (GQA + RoPE + QK-RMSNorm) on 8 trn2 NeuronCores.

Reference (B=2, T=2048, C=2048, 16 q-heads / 4 kv-heads, head_dim 128):
    q = rms_norm(rope(x @ Wq)) / sqrt(128); k = rms_norm(rope(x @ Wk))
    att = softmax_causal(q k^T / sqrt(128)); y = (att @ v) @ Wp

Sharding: core = 4*b + g  (b = batch 0..1, g = head-group 0..3).
Each core computes q-heads 4g..4g+3 (which all map to kv-head g under the
GQA repeat), attends over the full causal sequence of its batch, then the
4 cores of a batch AllGather their attention outputs and each computes a
distinct 512-column slice of the output projection. Host concatenates.

On-chip layout is "transposed activations": X^T, Q^T, K^T, S^T, P^T, Y^T,
OUT^T all [feature, t], so every matmul consumes its operands with the
contraction on the partition axis and no on-chip transposes are needed.
Softmax runs without max-subtraction: q and k are rms-normalized so
|q·k|/128 <= 1 and exp() cannot overflow.

All matmuls run in float32r (full PE rate; fp32 is 4x slower).
"""

import numpy as np

B, T, C = 2, 2048, 2048
NH, NKV, HD = 16, 4, 128
G = 4  # q-heads per core
EPS = 1e-6
NCB = C // 128  # 16 contraction blocks
NTCH = T // 512  # 4 t-chunks
NTKB = T // 128  # 16 key blocks

_CACHE = {}


def _build():
    import concourse.mybir as mybir
    import concourse.tile as tile
    from concourse import bacc
    from contextlib import ExitStack

    F32 = mybir.dt.float32
    F32R = mybir.dt.float32r
    AF = mybir.ActivationFunctionType

    nc = bacc.Bacc(None, target_bir_lowering=False, num_devices=8)

    xT = nc.dram_tensor("xT", [C, T], F32R, kind="ExternalInput")
    wq = nc.dram_tensor("wq", [C, G * HD], F32R, kind="ExternalInput")
    wk = nc.dram_tensor("wk", [C, HD], F32R, kind="ExternalInput")
    wv = nc.dram_tensor("wv", [C, HD], F32R, kind="ExternalInput")
    wp = nc.dram_tensor("wp", [C, G * HD], F32R, kind="ExternalInput")
    cosT = nc.dram_tensor("cosT", [64, T], F32, kind="ExternalInput")
    sinT = nc.dram_tensor("sinT", [64, T], F32, kind="ExternalInput")
    masks = nc.dram_tensor("masks", [4, 128, 512], F32, kind="ExternalInput")
    ones = nc.dram_tensor("ones", [128, 1], F32R, kind="ExternalInput")
    outT = nc.dram_tensor("outT", [G * HD, T], F32, kind="ExternalOutput")

    with tile.TileContext(nc) as tc:
        with ExitStack() as outer:
            dram = outer.enter_context(tc.tile_pool(name="dram", bufs=1, space="DRAM"))
            ag_in = dram.tile([G * HD, T], F32R)
            ag_out = dram.tile([4 * G * HD, T], F32R)

            consts = outer.enter_context(tc.tile_pool(name="consts", bufs=1))
            ones_sb = consts.tile([128, 1], F32R)
            nc.sync.dma_start(out=ones_sb[:], in_=ones[:])
            rk_col = consts.tile([128, NTKB], F32)
            eps_k = consts.tile([128, 1], F32)
            nc.vector.memset(eps_k[:], EPS)
            eps_q = consts.tile([1, 1], F32)
            nc.vector.memset(eps_q[:], float(HD * HD) * EPS)
            ones_f32 = consts.tile([128, 1], F32)
            nc.vector.memset(ones_f32[:], 1.0)

            with ExitStack() as sA:
                wpool = sA.enter_context(tc.tile_pool(name="w", bufs=1))
                wq_sb = wpool.tile([128, NCB, G * HD], F32R)
                wk_sb = wpool.tile([128, NCB, HD], F32R)
                wv_sb = wpool.tile([128, NCB, HD], F32R)
                nc.sync.dma_start(
                    out=wq_sb[:], in_=wq.rearrange("(cb p) m -> p cb m", p=128)
                )
                nc.sync.dma_start(
                    out=wk_sb[:], in_=wk.rearrange("(cb p) m -> p cb m", p=128)
                )
                nc.sync.dma_start(
                    out=wv_sb[:], in_=wv.rearrange("(cb p) m -> p cb m", p=128)
                )

                trig = sA.enter_context(tc.tile_pool(name="trig", bufs=1))
                cos_sb = trig.tile([128, T], F32)
                sin_sb = trig.tile([128, T], F32)
                masks_sb = trig.tile([128, 4, 512], F32)
                nc.sync.dma_start(out=cos_sb[0:64, :], in_=cosT[:])
                nc.sync.dma_start(out=cos_sb[64:128, :], in_=cosT[:])
                nc.sync.dma_start(out=sin_sb[64:128, :], in_=sinT[:])
                nc.scalar.mul(out=sin_sb[0:64, :], in_=sin_sb[64:128, :], mul=-1.0)
                nc.sync.dma_start(
                    out=masks_sb[:], in_=masks.rearrange("d p m -> p d m")
                )

                acts = sA.enter_context(tc.tile_pool(name="acts", bufs=1))
                qT_sb = acts.tile([128, G, T], F32R)
                kT_sb = acts.tile([128, T], F32R)
                v_sb = acts.tile([128, NTKB, HD], F32R)

                xt_pool = sA.enter_context(tc.tile_pool(name="xt", bufs=18))
                tmp = sA.enter_context(tc.tile_pool(name="tmp", bufs=2))
                small = sA.enter_context(tc.tile_pool(name="small", bufs=2))
                sP1 = ExitStack()
                psq = sP1.enter_context(tc.tile_pool(name="psq", bufs=4, space="PSUM"))
                psk = sP1.enter_context(tc.tile_pool(name="psk", bufs=1, space="PSUM"))
                psv = sP1.enter_context(tc.tile_pool(name="psv", bufs=1, space="PSUM"))
                pss = sP1.enter_context(tc.tile_pool(name="pss", bufs=1, space="PSUM"))
                psc = sP1.enter_context(tc.tile_pool(name="psc", bufs=1, space="PSUM"))

                def rope(dst_f32r, src_psum, tcs):
                    """dst = src*cos + rotate_half(src)*sin over t-cols tcs.
                    Returns the fp32 rotated tile for squaring."""
                    rot = tmp.tile([128, 512], F32, tag="rot")
                    nc.vector.tensor_copy(out=rot[0:64, :], in_=src_psum[64:128, :])
                    nc.vector.tensor_copy(out=rot[64:128, :], in_=src_psum[0:64, :])
                    qr = tmp.tile([128, 512], F32, tag="qr")
                    nc.vector.tensor_mul(qr[:], src_psum[:], cos_sb[:, tcs])
                    nc.vector.tensor_mul(rot[:], rot[:], sin_sb[:, tcs])
                    nc.vector.tensor_add(dst_f32r, qr[:], rot[:])

                # ---- phase 1: Q/K/V projections + RoPE + RMS-norm ----
                for tch in range(NTCH):
                    tcs = slice(512 * tch, 512 * tch + 512)
                    xts = []
                    for cb in range(NCB):
                        xt = xt_pool.tile([128, 512], F32R, tag="xt", name=f"xt{tch}_{cb}")
                        nc.sync.dma_start(
                            out=xt[:], in_=xT[128 * cb : 128 * cb + 128, tcs]
                        )
                        xts.append(xt)

                    # K^T chunk [128 (d), 512 (t)]
                    ps_k = psk.tile([128, 512], F32, tag="psk")
                    for cb in range(NCB):
                        nc.tensor.matmul(
                            ps_k[:], wk_sb[:, cb, :], xts[cb][:],
                            start=(cb == 0), stop=(cb == NCB - 1),
                        )
                    rope(kT_sb[:, tcs], ps_k[:], tcs)
                    ksq = tmp.tile([128, 512], F32, tag="sq")
                    nc.vector.tensor_mul(ksq[:], kT_sb[:, tcs], kT_sb[:, tcs])
                    for j in range(4):
                        ps_kc = psc.tile([128, 1], F32, tag="pskc")
                        nc.tensor.matmul(
                            ps_kc[:], ksq[:, 128 * j : 128 * j + 128], ones_f32[:],
                            start=True, stop=True,
                        )
                        scol = small.tile([128, 1], F32, tag="scol")
                        nc.scalar.activation(
                            out=scol[:], in_=ps_kc[:], func=AF.Sqrt,
                            scale=1.0 / HD, bias=eps_k[:],
                        )
                        nc.vector.reciprocal(
                            out=rk_col[:, 4 * tch + j : 4 * tch + j + 1], in_=scol[:]
                        )

                    # V natural [t, d], 4 sub-blocks of 128 t-rows
                    ps_v = psv.tile([128, 512], F32, tag="psv")
                    for tt in range(4):
                        for cb in range(NCB):
                            nc.tensor.matmul(
                                ps_v[:, 128 * tt : 128 * tt + 128],
                                xts[cb][:, 128 * tt : 128 * tt + 128],
                                wv_sb[:, cb, :],
                                start=(cb == 0), stop=(cb == NCB - 1),
                            )
                    for tt in range(4):
                        nc.scalar.activation(
                            out=v_sb[:, 4 * tch + tt, :],
                            in_=ps_v[:, 128 * tt : 128 * tt + 128],
                            func=AF.Copy,
                        )

                    # Q^T per head [128 (d), 512 (t)]
                    for hq in range(G):
                        ps_q = psq.tile([128, 512], F32, tag="psq")
                        for cb in range(NCB):
                            nc.tensor.matmul(
                                ps_q[:],
                                wq_sb[:, cb, 128 * hq : 128 * hq + 128],
                                xts[cb][:],
                                start=(cb == 0), stop=(cb == NCB - 1),
                            )
                        qrope = tmp.tile([128, 512], F32R, tag="qrope")
                        rope(qrope[:], ps_q[:], tcs)
                        sq = tmp.tile([128, 512], F32R, tag="sq")
                        nc.vector.tensor_mul(sq[:], qrope[:], qrope[:])
                        ps_ss = pss.tile([1, 512], F32, tag="psss")
                        nc.tensor.matmul(
                            ps_ss[:], ones_sb[:], sq[:], start=True, stop=True
                        )
                        # rq_eff = 1/(HD*sqrt(ms+eps)) = 1/sqrt(HD*ss + HD^2*eps)
                        srow = small.tile([1, 512], F32, tag="srow")
                        nc.scalar.activation(
                            out=srow[:], in_=ps_ss[:], func=AF.Sqrt,
                            scale=float(HD), bias=eps_q[:],
                        )
                        rrow = small.tile([1, 512], F32, tag="rrow")
                        nc.vector.reciprocal(out=rrow[:], in_=srow[:])
                        bc = tmp.tile([128, 512], F32, tag="bc")
                        nc.gpsimd.partition_broadcast(bc[:], rrow[:])
                        nc.vector.tensor_mul(
                            qT_sb[:, hq, tcs], qrope[:], bc[:]
                        )

                sP1.close()

                # ---- phase 2: causal attention, S^T/P^T orientation ----
                pt_pool = sA.enter_context(tc.tile_pool(name="pt", bufs=3))
                ps_s_pool = sA.enter_context(
                    tc.tile_pool(name="ps_s", bufs=3, space="PSUM")
                )
                ps_y_pool = sA.enter_context(
                    tc.tile_pool(name="ps_y", bufs=2, space="PSUM")
                )
                ps_rs_pool = sA.enter_context(
                    tc.tile_pool(name="ps_rs", bufs=2, space="PSUM")
                )

                for tqc in range(NTCH):
                    tqs = slice(512 * tqc, 512 * tqc + 512)
                    nblk = 4 * tqc + 4
                    for hq in range(G):
                        ps_y = ps_y_pool.tile([128, 512], F32, tag="psy")
                        ps_rs = ps_rs_pool.tile([1, 512], F32, tag="psrs")
                        for tkb in range(nblk):
                            ps_s = ps_s_pool.tile([128, 512], F32, tag="pss2")
                            nc.tensor.matmul(
                                ps_s[:],
                                kT_sb[:, 128 * tkb : 128 * tkb + 128],
                                qT_sb[:, hq, tqs],
                                start=True, stop=True,
                            )
                            pT = pt_pool.tile([128, 512], F32R, tag="pt")
                            nc.scalar.activation(
                                out=pT[:], in_=ps_s[:], func=AF.Exp,
                                scale=rk_col[:, tkb : tkb + 1],
                            )
                            d = tkb - 4 * tqc
                            if d >= 0:
                                nc.vector.tensor_mul(
                                    pT[:], pT[:], masks_sb[:, d, :]
                                )
                            nc.tensor.matmul(
                                ps_rs[:], ones_sb[:], pT[:],
                                start=(tkb == 0), stop=(tkb == nblk - 1),
                            )
                            nc.tensor.matmul(
                                ps_y[:], v_sb[:, tkb, :], pT[:],
                                start=(tkb == 0), stop=(tkb == nblk - 1),
                            )
                        rrow = small.tile([1, 512], F32, tag="rrow")
                        nc.vector.reciprocal(out=rrow[:], in_=ps_rs[:])
                        bc = tmp.tile([128, 512], F32, tag="bc")
                        nc.gpsimd.partition_broadcast(bc[:], rrow[:])
                        yT = tmp.tile([128, 512], F32R, tag="qrope")
                        nc.vector.tensor_mul(yT[:], ps_y[:], bc[:])
                        nc.sync.dma_start(
                            out=ag_in[128 * hq : 128 * hq + 128, tqs], in_=yT[:]
                        )

            # ---- phase 3: AllGather Y^T across the batch group ----
            nc.gpsimd.collective_compute(
                "AllGather",
                mybir.AluOpType.bypass,
                replica_groups=[[0, 1, 2, 3], [4, 5, 6, 7]],
                ins=[ag_in[:]],
                outs=[ag_out[:]],
            )

            # ---- phase 4: output projection (c_out slice) ----
            with ExitStack() as sB:
                wp_pool = sB.enter_context(tc.tile_pool(name="wpp", bufs=1))
                wp_sb = wp_pool.tile([128, NCB, G * HD], F32R)
                nc.sync.dma_start(
                    out=wp_sb[:], in_=wp.rearrange("(cb p) m -> p cb m", p=128)
                )
                yt_pool = sB.enter_context(tc.tile_pool(name="yt", bufs=20))
                ot_pool = sB.enter_context(tc.tile_pool(name="ot", bufs=3))
                ps_o_pool = sB.enter_context(
                    tc.tile_pool(name="ps_o", bufs=4, space="PSUM")
                )
                for tch in range(NTCH):
                    tcs = slice(512 * tch, 512 * tch + 512)
                    yts = []
                    for cb in range(NCB):
                        yt = yt_pool.tile(
                            [128, 512], F32R, tag="yt", name=f"yt{tch}_{cb}"
                        )
                        nc.sync.dma_start(
                            out=yt[:], in_=ag_out[128 * cb : 128 * cb + 128, tcs]
                        )
                        yts.append(yt)
                    for cob in range(4):
                        ps_o = ps_o_pool.tile([128, 512], F32, tag="pso")
                        for cb in range(NCB):
                            nc.tensor.matmul(
                                ps_o[:],
                                wp_sb[:, cb, 128 * cob : 128 * cob + 128],
                                yts[cb][:],
                                start=(cb == 0), stop=(cb == NCB - 1),
                            )
                        o_sb = ot_pool.tile([128, 512], F32, tag="osb")
                        nc.scalar.activation(
                            out=o_sb[:], in_=ps_o[:], func=AF.Copy
                        )
                        nc.sync.dma_start(
                            out=outT[128 * cob : 128 * cob + 128, tcs], in_=o_sb[:]
                        )

    nc.compile()
    return nc


def _get_nc():
    if "nc" not in _CACHE:
        _CACHE["nc"] = _build()
    return _CACHE["nc"]


def kernel(x, cos, sin, Wq, Wk, Wv, Wp):
    from concourse.bass_utils import run_bass_kernel_spmd

    x = np.asarray(x)
    f32 = np.float32
    cosT = np.ascontiguousarray(np.asarray(cos).T, dtype=f32)
    sinT = np.ascontiguousarray(np.asarray(sin).T, dtype=f32)
    ones = np.ones((128, 1), dtype=f32)
    p = np.arange(128, dtype=np.int64)[:, None]
    j = np.arange(512, dtype=np.int64)[None, :]
    masks = np.stack(
        [(j >= p + 128 * d).astype(f32) for d in range(4)], axis=0
    )  # [4, 128, 512]

    in_maps = []
    for core in range(8):
        b, g = core // 4, core % 4
        in_maps.append(
            {
                "xT": np.ascontiguousarray(x[b].T, dtype=f32),
                "wq": np.ascontiguousarray(
                    Wq[:, 512 * g : 512 * g + 512], dtype=f32
                ),
                "wk": np.ascontiguousarray(
                    Wk[:, 128 * g : 128 * g + 128], dtype=f32
                ),
                "wv": np.ascontiguousarray(
                    Wv[:, 128 * g : 128 * g + 128], dtype=f32
                ),
                "wp": np.ascontiguousarray(
                    Wp[:, 512 * g : 512 * g + 512], dtype=f32
                ),
                "cosT": cosT,
                "sinT": sinT,
                "masks": masks,
                "ones": ones,
            }
        )

    nc = _get_nc()
    res = run_bass_kernel_spmd(nc, in_maps, core_ids=list(range(8)), trace=False)

    out = np.empty((B, T, C), dtype=f32)
    for core in range(8):
        b, g = core // 4, core % 4
        out[b, :, 512 * g : 512 * g + 512] = res.results[core]["outT"].T
    return out


# revision 8
# speedup vs baseline: 1.3063x; 1.3063x over previous
"""Causal self-attention (GQA + RoPE + QK-RMSNorm) on 8 trn2 NeuronCores.

Reference (B=2, T=2048, C=2048, 16 q-heads / 4 kv-heads, head_dim 128):
    q = rms_norm(rope(x @ Wq)) / sqrt(128); k = rms_norm(rope(x @ Wk))
    att = softmax_causal(q k^T / sqrt(128)); y = (att @ v) @ Wp

Sharding: core = 4*b + g  (b = batch 0..1, g = head-group 0..3).
Each core computes q-heads 4g..4g+3 (which all map to kv-head g under the
GQA repeat), attends over the full causal sequence of its batch, then the
4 cores of a batch AllGather their attention outputs and each computes a
distinct 512-column slice of the output projection. Host concatenates.

On-chip layout is "transposed activations": X^T, Q^T, K^T, S^T, P^T, Y^T,
OUT^T all [feature, t], so every matmul consumes its operands with the
contraction on the partition axis and no on-chip transposes are needed.
Softmax runs without max-subtraction: q and k are rms-normalized so
|q·k|/128 <= 1 and exp() cannot overflow.

All matmuls run in float32r (full PE rate; fp32 is 4x slower).
"""

import ml_dtypes
import numpy as np

B, T, C = 2, 2048, 2048
NH, NKV, HD = 16, 4, 128
G = 4  # q-heads per core
EPS = 1e-6
NCB = C // 128  # 16 contraction blocks
NTCH = T // 512  # 4 t-chunks
NTKB = T // 128  # 16 key blocks

_CACHE = {}


def _build():
    import concourse.mybir as mybir
    import concourse.tile as tile
    from concourse import bacc
    from contextlib import ExitStack

    F32 = mybir.dt.float32
    F32R = mybir.dt.float32r
    BF16 = mybir.dt.bfloat16
    AF = mybir.ActivationFunctionType

    nc = bacc.Bacc(None, target_bir_lowering=False, num_devices=8)

    xT = nc.dram_tensor("xT", [C, T], F32R, kind="ExternalInput")
    wq = nc.dram_tensor("wq", [C, G * HD], F32R, kind="ExternalInput")
    wk = nc.dram_tensor("wk", [C, HD], F32R, kind="ExternalInput")
    wv = nc.dram_tensor("wv", [C, HD], F32R, kind="ExternalInput")
    wp = nc.dram_tensor("wp", [C, G * HD], BF16, kind="ExternalInput")
    cosT = nc.dram_tensor("cosT", [64, T], F32, kind="ExternalInput")
    sinT = nc.dram_tensor("sinT", [64, T], F32, kind="ExternalInput")
    masks = nc.dram_tensor("masks", [4, 128, 512], F32, kind="ExternalInput")
    ones = nc.dram_tensor("ones", [128, 1], F32R, kind="ExternalInput")
    outT = nc.dram_tensor("outT", [G * HD, T], F32, kind="ExternalOutput")

    with tile.TileContext(nc) as tc:
        with ExitStack() as outer:
            dram = outer.enter_context(tc.tile_pool(name="dram", bufs=1, space="DRAM"))
            ag_in = dram.tile([G * HD, T], BF16)
            ag_out_a = dram.tile([4 * 2 * HD, T], BF16)
            ag_out_b = dram.tile([4 * 2 * HD, T], BF16)

            consts = outer.enter_context(tc.tile_pool(name="consts", bufs=1))
            ones_sb = consts.tile([128, 1], F32R)
            nc.sync.dma_start(out=ones_sb[:], in_=ones[:])
            rk_col = consts.tile([128, NTKB], F32)
            eps_k = consts.tile([128, 1], F32)
            nc.vector.memset(eps_k[:], EPS)
            eps_q = consts.tile([1, 1], F32)
            nc.vector.memset(eps_q[:], float(HD * HD) * EPS)
            ones_f32 = consts.tile([128, 1], F32)
            nc.vector.memset(ones_f32[:], 1.0)

            with ExitStack() as sA:
                wpool = sA.enter_context(tc.tile_pool(name="w", bufs=1))
                wq_sb = wpool.tile([128, NCB, G * HD], F32R)
                wk_sb = wpool.tile([128, NCB, HD], F32R)
                wv_sb = wpool.tile([128, NCB, HD], F32R)
                nc.sync.dma_start(
                    out=wq_sb[:], in_=wq.rearrange("(cb p) m -> p cb m", p=128)
                )
                nc.sync.dma_start(
                    out=wk_sb[:], in_=wk.rearrange("(cb p) m -> p cb m", p=128)
                )
                nc.sync.dma_start(
                    out=wv_sb[:], in_=wv.rearrange("(cb p) m -> p cb m", p=128)
                )

                trig = sA.enter_context(tc.tile_pool(name="trig", bufs=1))
                cos_sb = trig.tile([128, T], F32)
                sin_sb = trig.tile([128, T], F32)
                masks_sb = trig.tile([128, 4, 512], F32)
                nc.sync.dma_start(out=cos_sb[0:64, :], in_=cosT[:])
                nc.sync.dma_start(out=cos_sb[64:128, :], in_=cosT[:])
                nc.sync.dma_start(out=sin_sb[64:128, :], in_=sinT[:])
                nc.vector.tensor_scalar_mul(sin_sb[0:64, :], sin_sb[64:128, :], -1.0)
                nc.sync.dma_start(
                    out=masks_sb[:], in_=masks.rearrange("d p m -> p d m")
                )

                acts = sA.enter_context(tc.tile_pool(name="acts", bufs=1))
                qT_sb = acts.tile([128, G, T], F32R)
                kT_sb = acts.tile([128, T], F32R)
                v_sb = acts.tile([128, NTKB, HD], F32R)

                xt_pool = sA.enter_context(tc.tile_pool(name="xt", bufs=18))
                tmp = sA.enter_context(tc.tile_pool(name="tmp", bufs=2))
                small = sA.enter_context(tc.tile_pool(name="small", bufs=2))
                sP1 = ExitStack()
                psq = sP1.enter_context(tc.tile_pool(name="psq", bufs=4, space="PSUM"))
                psk = sP1.enter_context(tc.tile_pool(name="psk", bufs=1, space="PSUM"))
                psv = sP1.enter_context(tc.tile_pool(name="psv", bufs=1, space="PSUM"))
                pss = sP1.enter_context(tc.tile_pool(name="pss", bufs=1, space="PSUM"))
                psc = sP1.enter_context(tc.tile_pool(name="psc", bufs=1, space="PSUM"))

                def rope(dst_f32r, src_psum, tcs):
                    """dst = src*cos + rotate_half(src)*sin over t-cols tcs.
                    Returns the fp32 rotated tile for squaring."""
                    rot = tmp.tile([128, 512], F32, tag="rot")
                    nc.vector.tensor_copy(out=rot[0:64, :], in_=src_psum[64:128, :])
                    nc.vector.tensor_copy(out=rot[64:128, :], in_=src_psum[0:64, :])
                    qr = tmp.tile([128, 512], F32, tag="qr")
                    nc.vector.tensor_mul(qr[:], src_psum[:], cos_sb[:, tcs])
                    nc.vector.tensor_mul(rot[:], rot[:], sin_sb[:, tcs])
                    nc.vector.tensor_add(dst_f32r, qr[:], rot[:])

                # ---- phase 1: Q/K/V projections + RoPE + RMS-norm ----
                for tch in range(NTCH):
                    tcs = slice(512 * tch, 512 * tch + 512)
                    xts = []
                    for cb in range(NCB):
                        xt = xt_pool.tile([128, 512], F32R, tag="xt", name=f"xt{tch}_{cb}")
                        nc.sync.dma_start(
                            out=xt[:], in_=xT[128 * cb : 128 * cb + 128, tcs]
                        )
                        xts.append(xt)

                    # K^T chunk [128 (d), 512 (t)]
                    ps_k = psk.tile([128, 512], F32, tag="psk")
                    for cb in range(NCB):
                        nc.tensor.matmul(
                            ps_k[:], wk_sb[:, cb, :], xts[cb][:],
                            start=(cb == 0), stop=(cb == NCB - 1),
                        )
                    rope(kT_sb[:, tcs], ps_k[:], tcs)
                    ksq = tmp.tile([128, 512], F32, tag="sq")
                    nc.vector.tensor_mul(ksq[:], kT_sb[:, tcs], kT_sb[:, tcs])
                    for j in range(4):
                        ps_kc = psc.tile([128, 1], F32, tag="pskc")
                        nc.tensor.matmul(
                            ps_kc[:], ksq[:, 128 * j : 128 * j + 128], ones_f32[:],
                            start=True, stop=True,
                        )
                        scol = small.tile([128, 1], F32, tag="scol")
                        nc.scalar.activation(
                            out=scol[:], in_=ps_kc[:], func=AF.Sqrt,
                            scale=1.0 / HD, bias=eps_k[:],
                        )
                        nc.vector.reciprocal(
                            out=rk_col[:, 4 * tch + j : 4 * tch + j + 1], in_=scol[:]
                        )

                    # V natural [t, d], 4 sub-blocks of 128 t-rows
                    ps_v = psv.tile([128, 512], F32, tag="psv")
                    for tt in range(4):
                        for cb in range(NCB):
                            nc.tensor.matmul(
                                ps_v[:, 128 * tt : 128 * tt + 128],
                                xts[cb][:, 128 * tt : 128 * tt + 128],
                                wv_sb[:, cb, :],
                                start=(cb == 0), stop=(cb == NCB - 1),
                            )
                    for tt in range(4):
                        nc.vector.tensor_copy(
                            out=v_sb[:, 4 * tch + tt, :],
                            in_=ps_v[:, 128 * tt : 128 * tt + 128],
                        )

                    # Q^T per head [128 (d), 512 (t)]
                    for hq in range(G):
                        ps_q = psq.tile([128, 512], F32, tag="psq")
                        for cb in range(NCB):
                            nc.tensor.matmul(
                                ps_q[:],
                                wq_sb[:, cb, 128 * hq : 128 * hq + 128],
                                xts[cb][:],
                                start=(cb == 0), stop=(cb == NCB - 1),
                            )
                        qrope = tmp.tile([128, 512], F32R, tag="qrope")
                        rope(qrope[:], ps_q[:], tcs)
                        sq = tmp.tile([128, 512], F32R, tag="sq")
                        nc.vector.tensor_mul(sq[:], qrope[:], qrope[:])
                        ps_ss = pss.tile([1, 512], F32, tag="psss")
                        nc.tensor.matmul(
                            ps_ss[:], ones_sb[:], sq[:], start=True, stop=True
                        )
                        # rq_eff = 1/(HD*sqrt(ms+eps)) = 1/sqrt(HD*ss + HD^2*eps)
                        srow = small.tile([1, 512], F32, tag="srow")
                        nc.scalar.activation(
                            out=srow[:], in_=ps_ss[:], func=AF.Sqrt,
                            scale=float(HD), bias=eps_q[:],
                        )
                        rrow = small.tile([1, 512], F32, tag="rrow")
                        nc.vector.reciprocal(out=rrow[:], in_=srow[:])
                        bc = tmp.tile([128, 512], F32, tag="bc")
                        nc.gpsimd.partition_broadcast(bc[:], rrow[:])
                        nc.vector.tensor_mul(
                            qT_sb[:, hq, tcs], qrope[:], bc[:]
                        )

                sP1.close()

                # ---- phase 2: causal attention, S^T/P^T orientation ----
                pt_pool = sA.enter_context(tc.tile_pool(name="pt", bufs=3))
                ps_s_pool = sA.enter_context(
                    tc.tile_pool(name="ps_s", bufs=4, space="PSUM")
                )
                ps_y_pool = sA.enter_context(
                    tc.tile_pool(name="ps_y", bufs=2, space="PSUM")
                )
                ps_rs_pool = sA.enter_context(
                    tc.tile_pool(name="ps_rs", bufs=2, space="PSUM")
                )

                for hq in range(G):
                    for tqc in range(NTCH):
                        tqs = slice(512 * tqc, 512 * tqc + 512)
                        nblk = 4 * tqc + 4
                        ps_y = ps_y_pool.tile([128, 512], F32, tag="psy")
                        ps_rs = ps_rs_pool.tile([1, 512], F32, tag="psrs")
                        for tkb in range(nblk):
                            ps_s = ps_s_pool.tile([128, 512], F32, tag="pss2")
                            nc.tensor.matmul(
                                ps_s[:],
                                kT_sb[:, 128 * tkb : 128 * tkb + 128],
                                qT_sb[:, hq, tqs],
                                start=True, stop=True,
                            )
                            pT = pt_pool.tile([128, 512], F32R, tag="pt")
                            nc.scalar.activation(
                                out=pT[:], in_=ps_s[:], func=AF.Exp,
                                scale=rk_col[:, tkb : tkb + 1],
                            )
                            d = tkb - 4 * tqc
                            if d >= 0:
                                nc.vector.tensor_mul(
                                    pT[:], pT[:], masks_sb[:, d, :]
                                )
                            nc.tensor.matmul(
                                ps_rs[:], ones_sb[:], pT[:],
                                start=(tkb == 0), stop=(tkb == nblk - 1),
                            )
                            nc.tensor.matmul(
                                ps_y[:], v_sb[:, tkb, :], pT[:],
                                start=(tkb == 0), stop=(tkb == nblk - 1),
                            )
                        rrow = small.tile([1, 512], F32, tag="rrow")
                        nc.vector.reciprocal(out=rrow[:], in_=ps_rs[:])
                        bc = tmp.tile([128, 512], F32, tag="bc")
                        nc.gpsimd.partition_broadcast(bc[:], rrow[:])
                        yT = tmp.tile([128, 512], BF16, tag="ybf")
                        nc.vector.tensor_mul(yT[:], ps_y[:], bc[:])
                        nc.sync.dma_start(
                            out=ag_in[128 * hq : 128 * hq + 128, tqs], in_=yT[:]
                        )
                    if hq == 1:
                        nc.gpsimd.collective_compute(
                            "AllGather",
                            mybir.AluOpType.bypass,
                            replica_groups=[[0, 1, 2, 3], [4, 5, 6, 7]],
                            ins=[ag_in[0 : 2 * HD, :]],
                            outs=[ag_out_a[:]],
                        )
                    if hq == 3:
                        nc.gpsimd.collective_compute(
                            "AllGather",
                            mybir.AluOpType.bypass,
                            replica_groups=[[0, 1, 2, 3], [4, 5, 6, 7]],
                            ins=[ag_in[2 * HD : 4 * HD, :]],
                            outs=[ag_out_b[:]],
                        )

            # ---- phase 4: output projection (c_out slice), bf16 ----
            # AG block i of a half maps to c_in block: rank r=i//2 owns global
            # heads 4r+hq; half A covers hq 0,1 and half B covers hq 2,3.
            with ExitStack() as sB:
                wp_pool = sB.enter_context(tc.tile_pool(name="wpp", bufs=1))
                wp_sb = wp_pool.tile([128, NCB, G * HD], BF16)
                nc.sync.dma_start(
                    out=wp_sb[:], in_=wp.rearrange("(cb p) m -> p cb m", p=128)
                )
                yt_pool = sB.enter_context(tc.tile_pool(name="yt", bufs=20))
                acc_pool = sB.enter_context(tc.tile_pool(name="acc", bufs=16))
                ot_pool = sB.enter_context(tc.tile_pool(name="ot", bufs=3))
                ps_o_pool = sB.enter_context(
                    tc.tile_pool(name="ps_o", bufs=4, space="PSUM")
                )
                accs = {}
                for half, ag_out_h in ((0, ag_out_a), (1, ag_out_b)):
                    cbs = [4 * (i // 2) + 2 * half + (i % 2) for i in range(8)]
                    for tch in range(NTCH):
                        tcs = slice(512 * tch, 512 * tch + 512)
                        yts = []
                        for i in range(8):
                            yt = yt_pool.tile(
                                [128, 512], BF16, tag="yt",
                                name=f"yt{half}_{tch}_{i}",
                            )
                            nc.sync.dma_start(
                                out=yt[:],
                                in_=ag_out_h[128 * i : 128 * i + 128, tcs],
                            )
                            yts.append(yt)
                        for cob in range(4):
                            ps_o = ps_o_pool.tile([128, 512], F32, tag="pso")
                            for i in range(8):
                                nc.tensor.matmul(
                                    ps_o[:],
                                    wp_sb[:, cbs[i], 128 * cob : 128 * cob + 128],
                                    yts[i][:],
                                    start=(i == 0), stop=(i == 7),
                                )
                            if half == 0:
                                acc = acc_pool.tile(
                                    [128, 512], F32, tag="acc",
                                    name=f"acc{tch}_{cob}",
                                )
                                nc.vector.tensor_copy(out=acc[:], in_=ps_o[:])
                                accs[(tch, cob)] = acc
                            else:
                                o_sb = ot_pool.tile([128, 512], F32, tag="osb")
                                nc.vector.tensor_add(
                                    o_sb[:], ps_o[:], accs[(tch, cob)][:]
                                )
                                nc.sync.dma_start(
                                    out=outT[128 * cob : 128 * cob + 128, tcs],
                                    in_=o_sb[:],
                                )

    nc.compile()
    return nc


def _get_nc():
    if "nc" not in _CACHE:
        _CACHE["nc"] = _build()
    return _CACHE["nc"]


def kernel(x, cos, sin, Wq, Wk, Wv, Wp):
    from concourse.bass_utils import run_bass_kernel_spmd

    x = np.asarray(x)
    f32 = np.float32
    cosT = np.ascontiguousarray(np.asarray(cos).T, dtype=f32)
    sinT = np.ascontiguousarray(np.asarray(sin).T, dtype=f32)
    ones = np.ones((128, 1), dtype=f32)
    p = np.arange(128, dtype=np.int64)[:, None]
    j = np.arange(512, dtype=np.int64)[None, :]
    masks = np.stack(
        [(j >= p + 128 * d).astype(f32) for d in range(4)], axis=0
    )  # [4, 128, 512]

    in_maps = []
    for core in range(8):
        b, g = core // 4, core % 4
        in_maps.append(
            {
                "xT": np.ascontiguousarray(x[b].T, dtype=f32),
                "wq": np.ascontiguousarray(
                    Wq[:, 512 * g : 512 * g + 512], dtype=f32
                ),
                "wk": np.ascontiguousarray(
                    Wk[:, 128 * g : 128 * g + 128], dtype=f32
                ),
                "wv": np.ascontiguousarray(
                    Wv[:, 128 * g : 128 * g + 128], dtype=f32
                ),
                "wp": np.ascontiguousarray(
                    Wp[:, 512 * g : 512 * g + 512]
                ).astype(ml_dtypes.bfloat16),
                "cosT": cosT,
                "sinT": sinT,
                "masks": masks,
                "ones": ones,
            }
        )

    nc = _get_nc()
    res = run_bass_kernel_spmd(nc, in_maps, core_ids=list(range(8)), trace=False)

    out = np.empty((B, T, C), dtype=f32)
    for core in range(8):
        b, g = core // 4, core % 4
        out[b, :, 512 * g : 512 * g + 512] = res.results[core]["outT"].T
    return out


# revision 10
# speedup vs baseline: 1.3545x; 1.0369x over previous
"""Causal self-attention (GQA + RoPE + QK-RMSNorm) on 8 trn2 NeuronCores.

Reference (B=2, T=2048, C=2048, 16 q-heads / 4 kv-heads, head_dim 128):
    q = rms_norm(rope(x @ Wq)) / sqrt(128); k = rms_norm(rope(x @ Wk))
    att = softmax_causal(q k^T / sqrt(128)); y = (att @ v) @ Wp

Sharding: core = 4*b + g  (b = batch 0..1, g = head-group 0..3).
Each core computes q-heads 4g..4g+3 (which all map to kv-head g under the
GQA repeat), attends over the full causal sequence of its batch, then the
4 cores of a batch AllGather their attention outputs and each computes a
distinct 512-column slice of the output projection. Host concatenates.

On-chip layout is "transposed activations": X^T, Q^T, K^T, S^T, P^T, Y^T,
OUT^T all [feature, t], so every matmul consumes its operands with the
contraction on the partition axis and no on-chip transposes are needed.
Softmax runs without max-subtraction: q and k are rms-normalized so
|q·k|/128 <= 1 and exp() cannot overflow.

All matmuls run in float32r (full PE rate; fp32 is 4x slower).
"""

import ml_dtypes
import numpy as np

B, T, C = 2, 2048, 2048
NH, NKV, HD = 16, 4, 128
G = 4  # q-heads per core
EPS = 1e-6
NCB = C // 128  # 16 contraction blocks
NTCH = T // 512  # 4 t-chunks
NTKB = T // 128  # 16 key blocks

_CACHE = {}


def _build():
    import concourse.mybir as mybir
    import concourse.tile as tile
    from concourse import bacc
    from contextlib import ExitStack

    F32 = mybir.dt.float32
    F32R = mybir.dt.float32r
    BF16 = mybir.dt.bfloat16
    AF = mybir.ActivationFunctionType

    nc = bacc.Bacc(None, target_bir_lowering=False, num_devices=8)

    xT = nc.dram_tensor("xT", [C, T], F32R, kind="ExternalInput")
    wq = nc.dram_tensor("wq", [C, G * HD], F32R, kind="ExternalInput")
    wk = nc.dram_tensor("wk", [C, HD], F32R, kind="ExternalInput")
    wv = nc.dram_tensor("wv", [C, HD], F32R, kind="ExternalInput")
    wp = nc.dram_tensor("wp", [C, G * HD], BF16, kind="ExternalInput")
    cosT = nc.dram_tensor("cosT", [64, T], F32, kind="ExternalInput")
    sinT = nc.dram_tensor("sinT", [64, T], F32, kind="ExternalInput")
    masks = nc.dram_tensor("masks", [4, 128, 512], BF16, kind="ExternalInput")
    ones = nc.dram_tensor("ones", [128, 1], F32R, kind="ExternalInput")
    outT = nc.dram_tensor("outT", [G * HD, T], F32, kind="ExternalOutput")

    with tile.TileContext(nc) as tc:
        with ExitStack() as outer:
            dram = outer.enter_context(tc.tile_pool(name="dram", bufs=1, space="DRAM"))
            ag_in = dram.tile([G * HD, T], BF16)
            ag_out_a = dram.tile([4 * 2 * HD, T], BF16)
            ag_out_b = dram.tile([4 * 2 * HD, T], BF16)

            consts = outer.enter_context(tc.tile_pool(name="consts", bufs=1))
            ones_sb = consts.tile([128, 1], F32R)
            nc.sync.dma_start(out=ones_sb[:], in_=ones[:])
            rk_col = consts.tile([128, NTKB], F32)
            eps_k = consts.tile([128, 1], F32)
            nc.vector.memset(eps_k[:], EPS)
            eps_q = consts.tile([1, 1], F32)
            nc.vector.memset(eps_q[:], float(HD * HD) * EPS)
            ones_f32 = consts.tile([128, 1], F32)
            nc.vector.memset(ones_f32[:], 1.0)
            ones_bf = consts.tile([128, 1], BF16)
            nc.vector.memset(ones_bf[:], 1.0)

            with ExitStack() as sA:
                wpool = sA.enter_context(tc.tile_pool(name="w", bufs=1))
                wq_sb = wpool.tile([128, NCB, G * HD], F32R)
                wk_sb = wpool.tile([128, NCB, HD], F32R)
                wv_sb = wpool.tile([128, NCB, HD], F32R)
                for cb in range(NCB):
                    nc.sync.dma_start(
                        out=wq_sb[:, cb, :], in_=wq[128 * cb : 128 * cb + 128, :]
                    )
                    nc.sync.dma_start(
                        out=wk_sb[:, cb, :], in_=wk[128 * cb : 128 * cb + 128, :]
                    )
                    nc.sync.dma_start(
                        out=wv_sb[:, cb, :], in_=wv[128 * cb : 128 * cb + 128, :]
                    )

                trig = sA.enter_context(tc.tile_pool(name="trig", bufs=1))
                cos_sb = trig.tile([128, T], F32)
                sin_sb = trig.tile([128, T], F32)
                masks_sb = trig.tile([128, 4, 512], BF16)
                nc.sync.dma_start(out=cos_sb[0:64, :], in_=cosT[:])
                nc.sync.dma_start(out=cos_sb[64:128, :], in_=cosT[:])
                nc.sync.dma_start(out=sin_sb[64:128, :], in_=sinT[:])
                nc.vector.tensor_scalar_mul(sin_sb[0:64, :], sin_sb[64:128, :], -1.0)
                nc.sync.dma_start(
                    out=masks_sb[:], in_=masks.rearrange("d p m -> p d m")
                )

                acts = sA.enter_context(tc.tile_pool(name="acts", bufs=1))
                qT_sb = acts.tile([128, G, T], F32R)
                kT_sb = acts.tile([128, T], F32R)
                v_sb = acts.tile([128, NTKB, HD], BF16)

                xt_pool = sA.enter_context(tc.tile_pool(name="xt", bufs=18))
                tmp = sA.enter_context(tc.tile_pool(name="tmp", bufs=2))
                small = sA.enter_context(tc.tile_pool(name="small", bufs=2))
                sP1 = ExitStack()
                psq = sP1.enter_context(tc.tile_pool(name="psq", bufs=4, space="PSUM"))
                psk = sP1.enter_context(tc.tile_pool(name="psk", bufs=1, space="PSUM"))
                psv = sP1.enter_context(tc.tile_pool(name="psv", bufs=1, space="PSUM"))
                pss = sP1.enter_context(tc.tile_pool(name="pss", bufs=1, space="PSUM"))
                psc = sP1.enter_context(tc.tile_pool(name="psc", bufs=1, space="PSUM"))

                def rope(dst_f32r, src_psum, tcs):
                    """dst = src*cos + rotate_half(src)*sin over t-cols tcs.
                    Returns the fp32 rotated tile for squaring."""
                    rot = tmp.tile([128, 512], F32, tag="rot")
                    nc.vector.tensor_copy(out=rot[0:64, :], in_=src_psum[64:128, :])
                    nc.vector.tensor_copy(out=rot[64:128, :], in_=src_psum[0:64, :])
                    qr = tmp.tile([128, 512], F32, tag="qr")
                    nc.vector.tensor_mul(qr[:], src_psum[:], cos_sb[:, tcs])
                    nc.vector.tensor_mul(rot[:], rot[:], sin_sb[:, tcs])
                    nc.vector.tensor_add(dst_f32r, qr[:], rot[:])

                # ---- phase 1: Q/K/V projections + RoPE + RMS-norm ----
                for tch in range(NTCH):
                    tcs = slice(512 * tch, 512 * tch + 512)
                    xts = []
                    for cb in range(NCB):
                        xt = xt_pool.tile([128, 512], F32R, tag="xt", name=f"xt{tch}_{cb}")
                        nc.sync.dma_start(
                            out=xt[:], in_=xT[128 * cb : 128 * cb + 128, tcs]
                        )
                        xts.append(xt)

                    # K^T chunk [128 (d), 512 (t)]
                    ps_k = psk.tile([128, 512], F32, tag="psk")
                    for cb in range(NCB):
                        nc.tensor.matmul(
                            ps_k[:], wk_sb[:, cb, :], xts[cb][:],
                            start=(cb == 0), stop=(cb == NCB - 1),
                        )
                    rope(kT_sb[:, tcs], ps_k[:], tcs)
                    ksq = tmp.tile([128, 512], F32, tag="sq")
                    nc.vector.tensor_mul(ksq[:], kT_sb[:, tcs], kT_sb[:, tcs])
                    for j in range(4):
                        ps_kc = psc.tile([128, 1], F32, tag="pskc")
                        nc.tensor.matmul(
                            ps_kc[:], ksq[:, 128 * j : 128 * j + 128], ones_f32[:],
                            start=True, stop=True,
                        )
                        scol = small.tile([128, 1], F32, tag="scol")
                        nc.scalar.activation(
                            out=scol[:], in_=ps_kc[:], func=AF.Sqrt,
                            scale=1.0 / HD, bias=eps_k[:],
                        )
                        nc.vector.reciprocal_approx_fast(
                            out=rk_col[:, 4 * tch + j : 4 * tch + j + 1], in_=scol[:]
                        )

                    # V natural [t, d], 4 sub-blocks of 128 t-rows
                    ps_v = psv.tile([128, 512], F32, tag="psv")
                    for tt in range(4):
                        for cb in range(NCB):
                            nc.tensor.matmul(
                                ps_v[:, 128 * tt : 128 * tt + 128],
                                xts[cb][:, 128 * tt : 128 * tt + 128],
                                wv_sb[:, cb, :],
                                start=(cb == 0), stop=(cb == NCB - 1),
                            )
                    for tt in range(4):
                        nc.vector.tensor_copy(
                            out=v_sb[:, 4 * tch + tt, :],
                            in_=ps_v[:, 128 * tt : 128 * tt + 128],
                        )

                    # Q^T per head [128 (d), 512 (t)]
                    for hq in range(G):
                        ps_q = psq.tile([128, 512], F32, tag="psq")
                        for cb in range(NCB):
                            nc.tensor.matmul(
                                ps_q[:],
                                wq_sb[:, cb, 128 * hq : 128 * hq + 128],
                                xts[cb][:],
                                start=(cb == 0), stop=(cb == NCB - 1),
                            )
                        qrope = tmp.tile([128, 512], F32R, tag="qrope")
                        rope(qrope[:], ps_q[:], tcs)
                        sq = tmp.tile([128, 512], F32R, tag="sq")
                        nc.vector.tensor_mul(sq[:], qrope[:], qrope[:])
                        ps_ss = pss.tile([1, 512], F32, tag="psss")
                        nc.tensor.matmul(
                            ps_ss[:], ones_sb[:], sq[:], start=True, stop=True
                        )
                        # rq_eff = 1/(HD*sqrt(ms+eps)) = 1/sqrt(HD*ss + HD^2*eps)
                        srow = small.tile([1, 512], F32, tag="srow")
                        nc.scalar.activation(
                            out=srow[:], in_=ps_ss[:], func=AF.Sqrt,
                            scale=float(HD), bias=eps_q[:],
                        )
                        rrow = small.tile([1, 512], F32, tag="rrow")
                        nc.vector.reciprocal_approx_fast(out=rrow[:], in_=srow[:])
                        bc = tmp.tile([128, 512], F32, tag="bc")
                        nc.gpsimd.partition_broadcast(bc[:], rrow[:])
                        nc.vector.tensor_mul(
                            qT_sb[:, hq, tcs], qrope[:], bc[:]
                        )

                sP1.close()

                # ---- phase 2: causal attention, S^T/P^T orientation ----
                pt_pool = sA.enter_context(tc.tile_pool(name="pt", bufs=3))
                ps_s_pool = sA.enter_context(
                    tc.tile_pool(name="ps_s", bufs=4, space="PSUM")
                )
                ps_y_pool = sA.enter_context(
                    tc.tile_pool(name="ps_y", bufs=2, space="PSUM")
                )
                ps_rs_pool = sA.enter_context(
                    tc.tile_pool(name="ps_rs", bufs=2, space="PSUM")
                )

                for hq in range(G):
                    for tqc in range(NTCH):
                        tqs = slice(512 * tqc, 512 * tqc + 512)
                        nblk = 4 * tqc + 4
                        ps_y = ps_y_pool.tile([128, 512], F32, tag="psy")
                        ps_rs = ps_rs_pool.tile([1, 512], F32, tag="psrs")
                        for tkb in range(nblk):
                            ps_s = ps_s_pool.tile([128, 512], F32, tag="pss2")
                            nc.tensor.matmul(
                                ps_s[:],
                                kT_sb[:, 128 * tkb : 128 * tkb + 128],
                                qT_sb[:, hq, tqs],
                                start=True, stop=True,
                            )
                            pT = pt_pool.tile([128, 512], BF16, tag="pt")
                            nc.scalar.activation(
                                out=pT[:], in_=ps_s[:], func=AF.Exp,
                                scale=rk_col[:, tkb : tkb + 1],
                            )
                            d = tkb - 4 * tqc
                            if d >= 0:
                                nc.vector.tensor_mul(
                                    pT[:], pT[:], masks_sb[:, d, :]
                                )
                            nc.tensor.matmul(
                                ps_rs[:], ones_bf[:], pT[:],
                                start=(tkb == 0), stop=(tkb == nblk - 1),
                            )
                            nc.tensor.matmul(
                                ps_y[:], v_sb[:, tkb, :], pT[:],
                                start=(tkb == 0), stop=(tkb == nblk - 1),
                            )
                        rrow = small.tile([1, 512], F32, tag="rrow")
                        nc.vector.reciprocal_approx_fast(out=rrow[:], in_=ps_rs[:])
                        bc = tmp.tile([128, 512], F32, tag="bc")
                        nc.gpsimd.partition_broadcast(bc[:], rrow[:])
                        yT = tmp.tile([128, 512], BF16, tag="ybf")
                        nc.vector.tensor_mul(yT[:], ps_y[:], bc[:])
                        nc.sync.dma_start(
                            out=ag_in[128 * hq : 128 * hq + 128, tqs], in_=yT[:]
                        )
                    if hq == 1:
                        nc.gpsimd.collective_compute(
                            "AllGather",
                            mybir.AluOpType.bypass,
                            replica_groups=[[0, 1, 2, 3], [4, 5, 6, 7]],
                            ins=[ag_in[0 : 2 * HD, :]],
                            outs=[ag_out_a[:]],
                        )
                    if hq == 3:
                        nc.gpsimd.collective_compute(
                            "AllGather",
                            mybir.AluOpType.bypass,
                            replica_groups=[[0, 1, 2, 3], [4, 5, 6, 7]],
                            ins=[ag_in[2 * HD : 4 * HD, :]],
                            outs=[ag_out_b[:]],
                        )

            # ---- phase 4: output projection (c_out slice), bf16 ----
            # AG block i of a half maps to c_in block: rank r=i//2 owns global
            # heads 4r+hq; half A covers hq 0,1 and half B covers hq 2,3.
            with ExitStack() as sB:
                wp_pool = sB.enter_context(tc.tile_pool(name="wpp", bufs=1))
                wp_sb = wp_pool.tile([128, NCB, G * HD], BF16)
                nc.sync.dma_start(
                    out=wp_sb[:], in_=wp.rearrange("(cb p) m -> p cb m", p=128)
                )
                yt_pool = sB.enter_context(tc.tile_pool(name="yt", bufs=20))
                acc_pool = sB.enter_context(tc.tile_pool(name="acc", bufs=16))
                ot_pool = sB.enter_context(tc.tile_pool(name="ot", bufs=3))
                ps_o_pool = sB.enter_context(
                    tc.tile_pool(name="ps_o", bufs=4, space="PSUM")
                )
                accs = {}
                for half, ag_out_h in ((0, ag_out_a), (1, ag_out_b)):
                    cbs = [4 * (i // 2) + 2 * half + (i % 2) for i in range(8)]
                    for tch in range(NTCH):
                        tcs = slice(512 * tch, 512 * tch + 512)
                        yts = []
                        for i in range(8):
                            yt = yt_pool.tile(
                                [128, 512], BF16, tag="yt",
                                name=f"yt{half}_{tch}_{i}",
                            )
                            nc.sync.dma_start(
                                out=yt[:],
                                in_=ag_out_h[128 * i : 128 * i + 128, tcs],
                            )
                            yts.append(yt)
                        for cob in range(4):
                            ps_o = ps_o_pool.tile([128, 512], F32, tag="pso")
                            for i in range(8):
                                nc.tensor.matmul(
                                    ps_o[:],
                                    wp_sb[:, cbs[i], 128 * cob : 128 * cob + 128],
                                    yts[i][:],
                                    start=(i == 0), stop=(i == 7),
                                )
                            if half == 0:
                                acc = acc_pool.tile(
                                    [128, 512], F32, tag="acc",
                                    name=f"acc{tch}_{cob}",
                                )
                                nc.vector.tensor_copy(out=acc[:], in_=ps_o[:])
                                accs[(tch, cob)] = acc
                            else:
                                o_sb = ot_pool.tile([128, 512], F32, tag="osb")
                                nc.vector.tensor_add(
                                    o_sb[:], ps_o[:], accs[(tch, cob)][:]
                                )
                                nc.sync.dma_start(
                                    out=outT[128 * cob : 128 * cob + 128, tcs],
                                    in_=o_sb[:],
                                )

    nc.compile()
    return nc


def _get_nc():
    if "nc" not in _CACHE:
        _CACHE["nc"] = _build()
    return _CACHE["nc"]


def kernel(x, cos, sin, Wq, Wk, Wv, Wp):
    from concourse.bass_utils import run_bass_kernel_spmd

    x = np.asarray(x)
    f32 = np.float32
    cosT = np.ascontiguousarray(np.asarray(cos).T, dtype=f32)
    sinT = np.ascontiguousarray(np.asarray(sin).T, dtype=f32)
    ones = np.ones((128, 1), dtype=f32)
    p = np.arange(128, dtype=np.int64)[:, None]
    j = np.arange(512, dtype=np.int64)[None, :]
    masks = np.stack(
        [(j >= p + 128 * d) for d in range(4)], axis=0
    ).astype(ml_dtypes.bfloat16)  # [4, 128, 512]

    in_maps = []
    for core in range(8):
        b, g = core // 4, core % 4
        in_maps.append(
            {
                "xT": np.ascontiguousarray(x[b].T, dtype=f32),
                "wq": np.ascontiguousarray(
                    Wq[:, 512 * g : 512 * g + 512], dtype=f32
                ),
                "wk": np.ascontiguousarray(
                    Wk[:, 128 * g : 128 * g + 128], dtype=f32
                ),
                "wv": np.ascontiguousarray(
                    Wv[:, 128 * g : 128 * g + 128], dtype=f32
                ),
                "wp": np.ascontiguousarray(
                    Wp[:, 512 * g : 512 * g + 512]
                ).astype(ml_dtypes.bfloat16),
                "cosT": cosT,
                "sinT": sinT,
                "masks": masks,
                "ones": ones,
            }
        )

    nc = _get_nc()
    res = run_bass_kernel_spmd(nc, in_maps, core_ids=list(range(8)), trace=False)

    out = np.empty((B, T, C), dtype=f32)
    for core in range(8):
        b, g = core // 4, core % 4
        out[b, :, 512 * g : 512 * g + 512] = res.results[core]["outT"].T
    return out


# revision 13
# speedup vs baseline: 1.5149x; 1.1184x over previous
"""Causal self-attention (GQA + RoPE + QK-RMSNorm) on 8 trn2 NeuronCores.

Reference (B=2, T=2048, C=2048, 16 q-heads / 4 kv-heads, head_dim 128):
    q = rms_norm(rope(x @ Wq)) / sqrt(128); k = rms_norm(rope(x @ Wk))
    att = softmax_causal(q k^T / sqrt(128)); y = (att @ v) @ Wp

Sharding: core = 4*b + g  (b = batch 0..1, g = head-group 0..3).
Each core computes q-heads 4g..4g+3 (which all map to kv-head g under the
GQA repeat), attends over the full causal sequence of its batch, then the
4 cores of a batch AllGather their attention outputs and each computes a
distinct 512-column slice of the output projection. Host concatenates.

On-chip layout is "transposed activations": X^T, Q^T, K^T, S^T, P^T, Y^T,
OUT^T all [feature, t], so every matmul consumes its operands with the
contraction on the partition axis and no on-chip transposes are needed.
Softmax runs without max-subtraction: q and k are rms-normalized so
|q·k|/128 <= 1 and exp() cannot overflow.

All matmuls run in float32r (full PE rate; fp32 is 4x slower).
"""

import ml_dtypes
import numpy as np

B, T, C = 2, 2048, 2048
NH, NKV, HD = 16, 4, 128
G = 4  # q-heads per core
EPS = 1e-6
NCB = C // 128  # 16 contraction blocks
NTCH = T // 512  # 4 t-chunks
NTKB = T // 128  # 16 key blocks

_CACHE = {}


def _build():
    import concourse.mybir as mybir
    import concourse.tile as tile
    from concourse import bacc
    from contextlib import ExitStack

    F32 = mybir.dt.float32
    F32R = mybir.dt.float32r
    BF16 = mybir.dt.bfloat16
    AF = mybir.ActivationFunctionType

    nc = bacc.Bacc(None, target_bir_lowering=False, num_devices=8)

    xT = nc.dram_tensor("xT", [C, T], F32R, kind="ExternalInput")
    wq = nc.dram_tensor("wq", [C, G * HD], F32R, kind="ExternalInput")
    wk = nc.dram_tensor("wk", [C, HD], F32R, kind="ExternalInput")
    wv = nc.dram_tensor("wv", [C, HD], F32R, kind="ExternalInput")
    wp = nc.dram_tensor("wp", [C, G * HD], BF16, kind="ExternalInput")
    cosT = nc.dram_tensor("cosT", [64, T], F32, kind="ExternalInput")
    sinT = nc.dram_tensor("sinT", [64, T], F32, kind="ExternalInput")
    masks = nc.dram_tensor("masks", [4, 128, 512], BF16, kind="ExternalInput")
    ones = nc.dram_tensor("ones", [128, 1], F32R, kind="ExternalInput")
    outT = nc.dram_tensor("outT", [G * HD, T], F32, kind="ExternalOutput")

    with tile.TileContext(nc) as tc:
        with ExitStack() as outer:
            dram = outer.enter_context(tc.tile_pool(name="dram", bufs=1, space="DRAM"))
            ag_in = dram.tile([G * HD, T], BF16)
            ag_outs = [
                dram.tile([4 * HD, T], BF16, name=f"ag_out_{q}") for q in range(4)
            ]

            consts = outer.enter_context(tc.tile_pool(name="consts", bufs=1))
            ones_sb = consts.tile([128, 1], F32R)
            nc.sync.dma_start(out=ones_sb[:], in_=ones[:])
            rk_col = consts.tile([128, NTKB], F32)
            eps_k = consts.tile([128, 1], F32)
            nc.vector.memset(eps_k[:], EPS)
            eps_q = consts.tile([1, 1], F32)
            nc.vector.memset(eps_q[:], float(HD * HD) * EPS)
            ones_f32 = consts.tile([128, 1], F32)
            nc.vector.memset(ones_f32[:], 1.0)
            ones_bf = consts.tile([128, 1], BF16)
            nc.vector.memset(ones_bf[:], 1.0)
            ident_bf = consts.tile([128, 128], BF16)
            from concourse.masks import make_identity
            make_identity(nc, ident_bf[:])

            with ExitStack() as sA:
                wpool = sA.enter_context(tc.tile_pool(name="w", bufs=1))
                wq_sb = wpool.tile([128, NCB, G * HD], F32R)
                wk_sb = wpool.tile([128, NCB, HD], F32R)
                wv_sb = wpool.tile([128, NCB, HD], F32R)
                for cb in range(NCB):
                    nc.sync.dma_start(
                        out=wk_sb[:, cb, :], in_=wk[128 * cb : 128 * cb + 128, :]
                    )
                for cb in range(NCB):
                    nc.sync.dma_start(
                        out=wv_sb[:, cb, :], in_=wv[128 * cb : 128 * cb + 128, :]
                    )
                for cb in range(NCB):
                    nc.sync.dma_start(
                        out=wq_sb[:, cb, :], in_=wq[128 * cb : 128 * cb + 128, :]
                    )

                trig = sA.enter_context(tc.tile_pool(name="trig", bufs=1))
                cos_sb = trig.tile([128, T], F32)
                sin_sb = trig.tile([128, T], F32)
                masks_sb = trig.tile([128, 4, 512], BF16)
                nc.sync.dma_start(out=cos_sb[0:64, :], in_=cosT[:])
                nc.sync.dma_start(out=cos_sb[64:128, :], in_=cosT[:])
                nc.sync.dma_start(out=sin_sb[64:128, :], in_=sinT[:])
                nc.vector.tensor_scalar_mul(sin_sb[0:64, :], sin_sb[64:128, :], -1.0)
                nc.sync.dma_start(
                    out=masks_sb[:], in_=masks.rearrange("d p m -> p d m")
                )

                acts = sA.enter_context(tc.tile_pool(name="acts", bufs=1))
                qT_sb = acts.tile([128, G, T], F32R)
                kT_sb = acts.tile([128, T], F32R)
                v_sb = acts.tile([128, NTKB, HD], BF16)

                xt_pool = sA.enter_context(tc.tile_pool(name="xt", bufs=18))
                tmp = sA.enter_context(tc.tile_pool(name="tmp", bufs=2))
                small = sA.enter_context(tc.tile_pool(name="small", bufs=2))
                sP1 = ExitStack()
                psq = sP1.enter_context(tc.tile_pool(name="psq", bufs=2, space="PSUM"))
                psk = sP1.enter_context(tc.tile_pool(name="psk", bufs=1, space="PSUM"))
                psv = sP1.enter_context(tc.tile_pool(name="psv", bufs=1, space="PSUM"))
                pss = sP1.enter_context(tc.tile_pool(name="pss", bufs=2, space="PSUM"))
                psc = pss
                pstr = sP1.enter_context(tc.tile_pool(name="pstr", bufs=2, space="PSUM"))

                def rope(dst_f32r, src_psum, tcs):
                    """dst = src*cos + rotate_half(src)*sin over t-cols tcs.
                    Returns the fp32 rotated tile for squaring."""
                    rot = tmp.tile([128, 512], F32, tag="rot")
                    nc.vector.tensor_copy(out=rot[0:64, :], in_=src_psum[64:128, :])
                    nc.vector.tensor_copy(out=rot[64:128, :], in_=src_psum[0:64, :])
                    qr = tmp.tile([128, 512], F32, tag="qr")
                    nc.vector.tensor_mul(qr[:], src_psum[:], cos_sb[:, tcs])
                    nc.vector.tensor_mul(rot[:], rot[:], sin_sb[:, tcs])
                    nc.vector.tensor_add(dst_f32r, qr[:], rot[:])

                # ---- phase 1: Q/K/V projections + RoPE + RMS-norm ----
                for tch in range(NTCH):
                    tcs = slice(512 * tch, 512 * tch + 512)
                    xts = []
                    for cb in range(NCB):
                        xt = xt_pool.tile([128, 512], F32R, tag="xt", name=f"xt{tch}_{cb}")
                        dma_eng = nc.scalar if tch == 0 else nc.sync
                        dma_eng.dma_start(
                            out=xt[:], in_=xT[128 * cb : 128 * cb + 128, tcs]
                        )
                        xts.append(xt)

                    # K^T chunk [128 (d), 512 (t)]
                    ps_k = psk.tile([128, 512], F32, tag="psk")
                    for cb in range(NCB):
                        nc.tensor.matmul(
                            ps_k[:], wk_sb[:, cb, :], xts[cb][:],
                            start=(cb == 0), stop=(cb == NCB - 1),
                        )
                    rope(kT_sb[:, tcs], ps_k[:], tcs)
                    ksq = tmp.tile([128, 512], F32, tag="sq")
                    nc.vector.tensor_mul(ksq[:], kT_sb[:, tcs], kT_sb[:, tcs])
                    for j in range(4):
                        ps_kc = psc.tile([128, 1], F32, tag="ssp")
                        nc.tensor.matmul(
                            ps_kc[:], ksq[:, 128 * j : 128 * j + 128], ones_f32[:],
                            start=True, stop=True,
                        )
                        scol = small.tile([128, 1], F32, tag="scol")
                        nc.scalar.activation(
                            out=scol[:], in_=ps_kc[:], func=AF.Sqrt,
                            scale=1.0 / HD, bias=eps_k[:],
                        )
                        nc.vector.reciprocal_approx_fast(
                            out=rk_col[:, 4 * tch + j : 4 * tch + j + 1], in_=scol[:]
                        )

                    # V^T [d, t], then PE-transpose each 128-block into v_sb
                    ps_v = psv.tile([128, 512], F32, tag="psv")
                    for cb in range(NCB):
                        nc.tensor.matmul(
                            ps_v[:], wv_sb[:, cb, :], xts[cb][:],
                            start=(cb == 0), stop=(cb == NCB - 1),
                        )
                    vt_bf = tmp.tile([128, 512], BF16, tag="vtb")
                    nc.vector.tensor_copy(out=vt_bf[:], in_=ps_v[:])
                    for tt in range(4):
                        ps_tr = pstr.tile([128, 128], BF16, tag="pstr")
                        nc.tensor.transpose(
                            ps_tr[:], vt_bf[:, 128 * tt : 128 * tt + 128],
                            ident_bf[:],
                        )
                        nc.vector.tensor_copy(
                            out=v_sb[:, 4 * tch + tt, :], in_=ps_tr[:]
                        )

                    # Q^T per head [128 (d), 512 (t)]
                    for hq in range(G):
                        ps_q = psq.tile([128, 512], F32, tag="psq")
                        for cb in range(NCB):
                            nc.tensor.matmul(
                                ps_q[:],
                                wq_sb[:, cb, 128 * hq : 128 * hq + 128],
                                xts[cb][:],
                                start=(cb == 0), stop=(cb == NCB - 1),
                            )
                        qrope = tmp.tile([128, 512], F32R, tag="qrope")
                        rope(qrope[:], ps_q[:], tcs)
                        sq = tmp.tile([128, 512], F32R, tag="sq")
                        nc.vector.tensor_mul(sq[:], qrope[:], qrope[:])
                        ps_ss = pss.tile([1, 512], F32, tag="ssp")
                        nc.tensor.matmul(
                            ps_ss[:], ones_sb[:], sq[:], start=True, stop=True
                        )
                        # rq_eff = 1/(HD*sqrt(ms+eps)) = 1/sqrt(HD*ss + HD^2*eps)
                        srow = small.tile([1, 512], F32, tag="srow")
                        nc.scalar.activation(
                            out=srow[:], in_=ps_ss[:], func=AF.Sqrt,
                            scale=float(HD), bias=eps_q[:],
                        )
                        rrow = small.tile([1, 512], F32, tag="rrow")
                        nc.vector.reciprocal_approx_fast(out=rrow[:], in_=srow[:])
                        bc = tmp.tile([128, 512], F32, tag="bc")
                        nc.gpsimd.partition_broadcast(bc[:], rrow[:])
                        nc.vector.tensor_mul(
                            qT_sb[:, hq, tcs], qrope[:], bc[:]
                        )

                sP1.close()

                # ---- phase 2: causal attention, S^T/P^T orientation ----
                pt_pool = sA.enter_context(tc.tile_pool(name="pt", bufs=3))
                ps_s_pool = sA.enter_context(
                    tc.tile_pool(name="ps_s", bufs=4, space="PSUM")
                )
                ps_y_pool = sA.enter_context(
                    tc.tile_pool(name="ps_y", bufs=2, space="PSUM")
                )
                ps_rs_pool = sA.enter_context(
                    tc.tile_pool(name="ps_rs", bufs=2, space="PSUM")
                )

                for hq in range(G):
                    for tqc in range(NTCH):
                        tqs = slice(512 * tqc, 512 * tqc + 512)
                        nblk = 4 * tqc + 4
                        ps_y = ps_y_pool.tile([128, 512], F32, tag="psy")
                        ps_rs = ps_rs_pool.tile([1, 512], F32, tag="psrs")
                        for tkb in range(nblk):
                            ps_s = ps_s_pool.tile([128, 512], F32, tag="pss2")
                            nc.tensor.matmul(
                                ps_s[:],
                                kT_sb[:, 128 * tkb : 128 * tkb + 128],
                                qT_sb[:, hq, tqs],
                                start=True, stop=True,
                            )
                            pT = pt_pool.tile([128, 512], BF16, tag="pt")
                            nc.scalar.activation(
                                out=pT[:], in_=ps_s[:], func=AF.Exp,
                                scale=rk_col[:, tkb : tkb + 1],
                            )
                            d = tkb - 4 * tqc
                            if d >= 0:
                                nc.vector.tensor_mul(
                                    pT[:], pT[:], masks_sb[:, d, :]
                                )
                            nc.tensor.matmul(
                                ps_rs[:], ones_bf[:], pT[:],
                                start=(tkb == 0), stop=(tkb == nblk - 1),
                            )
                            nc.tensor.matmul(
                                ps_y[:], v_sb[:, tkb, :], pT[:],
                                start=(tkb == 0), stop=(tkb == nblk - 1),
                            )
                        rrow = small.tile([1, 512], F32, tag="rrow")
                        nc.vector.reciprocal_approx_fast(out=rrow[:], in_=ps_rs[:])
                        bc = tmp.tile([128, 512], F32, tag="bc")
                        nc.gpsimd.partition_broadcast(bc[:], rrow[:])
                        yT = tmp.tile([128, 512], BF16, tag="ybf")
                        nc.vector.tensor_mul(yT[:], ps_y[:], bc[:])
                        nc.sync.dma_start(
                            out=ag_in[128 * hq : 128 * hq + 128, tqs], in_=yT[:]
                        )
                    nc.gpsimd.collective_compute(
                        "AllGather",
                        mybir.AluOpType.bypass,
                        replica_groups=[[0, 1, 2, 3], [4, 5, 6, 7]],
                        ins=[ag_in[HD * hq : HD * hq + HD, :]],
                        outs=[ag_outs[hq][:]],
                    )

            # ---- phase 4: output projection (c_out slice), bf16 ----
            # Quarter q gathers head hq=q of each rank r; its AG-out block r
            # corresponds to c_in block (global head) 4*r + q.
            with ExitStack() as sB:
                wp_pool = sB.enter_context(tc.tile_pool(name="wpp", bufs=1))
                wp_sb = wp_pool.tile([128, NCB, G * HD], BF16)
                for cb in range(NCB):
                    nc.sync.dma_start(
                        out=wp_sb[:, cb, :], in_=wp[128 * cb : 128 * cb + 128, :]
                    )
                yt_pool = sB.enter_context(tc.tile_pool(name="yt", bufs=10))
                acc_pool = sB.enter_context(tc.tile_pool(name="acc", bufs=16))
                ot_pool = sB.enter_context(tc.tile_pool(name="ot", bufs=3))
                ps_o_pool = sB.enter_context(
                    tc.tile_pool(name="ps_o", bufs=4, space="PSUM")
                )
                accs = {}
                for q in range(4):
                    for tch in range(NTCH):
                        tcs = slice(512 * tch, 512 * tch + 512)
                        yts = []
                        for r in range(4):
                            yt = yt_pool.tile(
                                [128, 512], BF16, tag="yt",
                                name=f"yt{q}_{tch}_{r}",
                            )
                            nc.sync.dma_start(
                                out=yt[:],
                                in_=ag_outs[q][128 * r : 128 * r + 128, tcs],
                            )
                            yts.append(yt)
                        for cob in range(4):
                            ps_o = ps_o_pool.tile([128, 512], F32, tag="pso")
                            for r in range(4):
                                nc.tensor.matmul(
                                    ps_o[:],
                                    wp_sb[:, 4 * r + q, 128 * cob : 128 * cob + 128],
                                    yts[r][:],
                                    start=(r == 0), stop=(r == 3),
                                )
                            if q == 0:
                                acc = acc_pool.tile(
                                    [128, 512], F32, tag="acc",
                                    name=f"acc{tch}_{cob}",
                                )
                                nc.vector.tensor_copy(out=acc[:], in_=ps_o[:])
                                accs[(tch, cob)] = acc
                            elif q < 3:
                                nc.vector.tensor_add(
                                    accs[(tch, cob)][:], accs[(tch, cob)][:],
                                    ps_o[:],
                                )
                            else:
                                o_sb = ot_pool.tile([128, 512], F32, tag="osb")
                                nc.vector.tensor_add(
                                    o_sb[:], ps_o[:], accs[(tch, cob)][:]
                                )
                                nc.sync.dma_start(
                                    out=outT[128 * cob : 128 * cob + 128, tcs],
                                    in_=o_sb[:],
                                )

    nc.compile()
    return nc


def _get_nc():
    if "nc" not in _CACHE:
        _CACHE["nc"] = _build()
    return _CACHE["nc"]


def kernel(x, cos, sin, Wq, Wk, Wv, Wp):
    from concourse.bass_utils import run_bass_kernel_spmd

    x = np.asarray(x)
    f32 = np.float32
    cosT = np.ascontiguousarray(np.asarray(cos).T, dtype=f32)
    sinT = np.ascontiguousarray(np.asarray(sin).T, dtype=f32)
    ones = np.ones((128, 1), dtype=f32)
    p = np.arange(128, dtype=np.int64)[:, None]
    j = np.arange(512, dtype=np.int64)[None, :]
    masks = np.stack(
        [(j >= p + 128 * d) for d in range(4)], axis=0
    ).astype(ml_dtypes.bfloat16)  # [4, 128, 512]

    in_maps = []
    for core in range(8):
        b, g = core // 4, core % 4
        in_maps.append(
            {
                "xT": np.ascontiguousarray(x[b].T, dtype=f32),
                "wq": np.ascontiguousarray(
                    Wq[:, 512 * g : 512 * g + 512], dtype=f32
                ),
                "wk": np.ascontiguousarray(
                    Wk[:, 128 * g : 128 * g + 128], dtype=f32
                ),
                "wv": np.ascontiguousarray(
                    Wv[:, 128 * g : 128 * g + 128], dtype=f32
                ),
                "wp": np.ascontiguousarray(
                    Wp[:, 512 * g : 512 * g + 512]
                ).astype(ml_dtypes.bfloat16),
                "cosT": cosT,
                "sinT": sinT,
                "masks": masks,
                "ones": ones,
            }
        )

    nc = _get_nc()
    res = run_bass_kernel_spmd(nc, in_maps, core_ids=list(range(8)), trace=False)

    out = np.empty((B, T, C), dtype=f32)
    for core in range(8):
        b, g = core // 4, core % 4
        out[b, :, 512 * g : 512 * g + 512] = res.results[core]["outT"].T
    return out
